# revision 8
# baseline (speedup 1.0000x reference)
"""GQA kernel for Trainium2, 8 NeuronCores.

Sharding: core c = b*4 + g  handles batch b, kv-head g (4 query heads).
Each core computes:
  Q_g^T = Wq_g @ x_q^T        [4 heads][128, S]   (scale 1/sqrt(D) folded in)
  K_g^T = Wk_g @ x_k^T        [128, S]
  V_g   = (x_v @ Wv_g.T)      [S, 128]  (via V^T then PE transpose)
  S^T   = K tile @ Q^T        [k,q] orientation -> +mask (diag) -> exp
  o^T  += V[kt] matmul P~^T   (PSUM accum), l += ones^T P~^T
  o_norm^T = o^T * recip(bcast l)
  partial = o_norm @ Wo_g.T   [S, E]
Host sums the 4 partials per batch.

Matmuls run in bf16 (fp32 PSUM accumulation): 4-byte dtypes serialize
LDWEIGHTS with the matmul (~191ns per 128x128 load, no FWL/prefetch),
which was ~37% of the kernel span in fp32r. l is broadcast across
partitions with a K=1 matmul so the reciprocal runs at full DVE lane
width ([128,512] not [1,512]).
"""

import sys

import numpy as np

for _p in ("/opt/trn_rl_repo",):
    if _p not in sys.path:
        sys.path.insert(0, _p)

import ml_dtypes

import concourse.bass as bass
import concourse.mybir as mybir
from concourse import bacc
from concourse.bass_utils import run_bass_kernel_spmd
from concourse.masks import make_identity
from concourse.tile import TileContext

B, S, E = 2, 2048, 2048
H, HKV = 16, 4
D = E // H  # 128
G = H // HKV  # 4 query heads per kv head
GD = G * D  # 512
NCORES = B * HKV  # 8
SC = 512  # s/q chunk width (free dim of matmuls)
NSC = S // SC  # 4
NET = E // 128  # 16 e-tiles (contraction)
NKT = S // 128  # 16 k-tiles
SCALE = 1.0 / float(np.sqrt(D))

F32 = mybir.dt.float32
BF16 = mybir.dt.bfloat16
AF = mybir.ActivationFunctionType
NPBF = np.dtype(ml_dtypes.bfloat16)


def build_nc():
    nc = bacc.Bacc()
    xq = nc.declare_dram_parameter("xq", [E, S], BF16, isOutput=False)  # query[b].T
    xk = nc.declare_dram_parameter("xk", [E, S], BF16, isOutput=False)  # key[b].T
    xv = nc.declare_dram_parameter("xv", [E, S], BF16, isOutput=False)  # value[b].T
    wq = nc.declare_dram_parameter("wq", [E, GD], BF16, isOutput=False)
    wk = nc.declare_dram_parameter("wk", [E, D], BF16, isOutput=False)
    wv = nc.declare_dram_parameter("wv", [E, D], BF16, isOutput=False)
    wo = nc.declare_dram_parameter("wo", [GD, E], BF16, isOutput=False)
    msk = nc.declare_dram_parameter("msk", [4 * 128, SC], F32, isOutput=False)
    out = nc.declare_dram_parameter("out", [S, E], F32, isOutput=True)

    with TileContext(nc) as tc:
        with (
            tc.tile_pool(name="singles", bufs=1) as singles,
            tc.tile_pool(name="xt", bufs=4) as xtp,
            tc.tile_pool(name="pexp", bufs=4) as pexp,
            tc.tile_pool(name="small", bufs=2) as small,
            tc.tile_pool(name="ob", bufs=3) as obp,
            tc.tile_pool(name="acc", bufs=4, space="PSUM") as acc,
            tc.tile_pool(name="ops", bufs=2, space="PSUM") as ops,
            tc.tile_pool(name="lps", bufs=1, space="PSUM") as lps,
            tc.tile_pool(name="trp", bufs=1, space="PSUM") as trp,
        ):
            # ---- constants / weights resident in SBUF ----
            wq_sb = singles.tile([128, NET, GD], BF16)  # 16KB/p
            wk_sb = singles.tile([128, NET, D], BF16)  # 4KB/p
            wv_sb = singles.tile([128, NET, D], BF16)  # 4KB/p
            wo_sb = singles.tile([128, G, E], BF16)  # 16KB/p
            mask_sb = singles.tile([128, 4, SC], F32)  # 8KB/p
            ident_f = singles.tile([128, 128], F32)
            ident = singles.tile([128, 128], BF16)
            ones_f = singles.tile([128, 1], F32)
            ones = singles.tile([128, 1], BF16)
            onesr_f = singles.tile([1, 128], F32)
            onesr = singles.tile([1, 128], BF16)
            qT = singles.tile([128, G, S], BF16)  # 16KB/p
            kT = singles.tile([128, S], BF16)  # 4KB/p
            v_sb = singles.tile([128, NKT, D], BF16)  # 4KB/p
            onrm = singles.tile([128, G, S], BF16)  # 16KB/p

            make_identity(nc, ident_f)
            nc.scalar.activation(out=ident[:], in_=ident_f[:], func=AF.Copy)
            nc.vector.memset(ones_f, 1.0)
            nc.scalar.activation(out=ones[:], in_=ones_f[:], func=AF.Copy)
            nc.vector.memset(onesr_f, 1.0)
            nc.scalar.activation(out=onesr[:], in_=onesr_f[:], func=AF.Copy)
            for t in range(NET):
                nc.sync.dma_start(out=wq_sb[:, t, :], in_=wq[t * 128 : (t + 1) * 128, :])
                nc.sync.dma_start(out=wk_sb[:, t, :], in_=wk[t * 128 : (t + 1) * 128, :])
                nc.sync.dma_start(out=wv_sb[:, t, :], in_=wv[t * 128 : (t + 1) * 128, :])
            for h in range(G):
                nc.sync.dma_start(
                    out=wo_sb[:, h, :], in_=wo[h * 128 : (h + 1) * 128, :]
                )
            for j in range(4):
                nc.sync.dma_start(
                    out=mask_sb[:, j, :], in_=msk[j * 128 : (j + 1) * 128, :]
                )

            # ---- phase 1: projections ----
            for sc in range(NSC):
                ssl = slice(sc * SC, (sc + 1) * SC)
                # Q^T: 4 heads
                xts = []
                for t in range(NET):
                    xt = xtp.tile([128, SC], BF16, tag="xt")
                    nc.sync.dma_start(out=xt, in_=xq[t * 128 : (t + 1) * 128, ssl])
                    xts.append(xt)
                for h in range(G):
                    ps = acc.tile([128, SC], F32, tag="acc")
                    for t in range(NET):
                        nc.tensor.matmul(
                            ps[:],
                            lhsT=wq_sb[:, t, h * D : (h + 1) * D],
                            rhs=xts[t][:],
                            start=(t == 0),
                            stop=(t == NET - 1),
                        )
                    # fold softmax scale into Q
                    nc.scalar.activation(
                        out=qT[:, h, ssl], in_=ps[:], func=AF.Copy, scale=SCALE
                    )
                # K^T
                xts = []
                for t in range(NET):
                    xt = xtp.tile([128, SC], BF16, tag="xt")
                    nc.sync.dma_start(out=xt, in_=xk[t * 128 : (t + 1) * 128, ssl])
                    xts.append(xt)
                ps = acc.tile([128, SC], F32, tag="acc")
                for t in range(NET):
                    nc.tensor.matmul(
                        ps[:],
                        lhsT=wk_sb[:, t, :],
                        rhs=xts[t][:],
                        start=(t == 0),
                        stop=(t == NET - 1),
                    )
                nc.vector.tensor_copy(out=kT[:, ssl], in_=ps[:])
                # V^T then transpose to V [s, d]
                xts = []
                for t in range(NET):
                    xt = xtp.tile([128, SC], BF16, tag="xt")
                    nc.sync.dma_start(out=xt, in_=xv[t * 128 : (t + 1) * 128, ssl])
                    xts.append(xt)
                ps = acc.tile([128, SC], F32, tag="acc")
                for t in range(NET):
                    nc.tensor.matmul(
                        ps[:],
                        lhsT=wv_sb[:, t, :],
                        rhs=xts[t][:],
                        start=(t == 0),
                        stop=(t == NET - 1),
                    )
                vt_tmp = small.tile([128, SC], BF16, tag="vt")
                nc.scalar.activation(out=vt_tmp[:], in_=ps[:], func=AF.Copy)
                for i in range(SC // 128):
                    tp = trp.tile([128, 128], BF16, tag="tr")
                    nc.tensor.transpose(
                        tp[:], vt_tmp[:, i * 128 : (i + 1) * 128], ident[:]
                    )
                    nc.vector.tensor_copy(out=v_sb[:, sc * 4 + i, :], in_=tp[:])

            # ---- phase 2: attention (flash, S^T orientation) ----
            for h in range(G):
                for qc in range(NSC):
                    qsl = slice(qc * SC, (qc + 1) * SC)
                    nkt = (qc + 1) * (SC // 128)  # causal: k tiles 0..nkt-1
                    o_ps = ops.tile([128, SC], F32, tag="o")
                    l_ps = lps.tile([1, SC], F32, tag="l")
                    for kt in range(nkt):
                        s_ps = acc.tile([128, SC], F32, tag="acc")
                        nc.tensor.matmul(
                            s_ps[:],
                            lhsT=kT[:, kt * 128 : (kt + 1) * 128],
                            rhs=qT[:, h, qsl],
                            start=True,
                            stop=True,
                        )
                        if kt >= nkt - 4:
                            j = kt - 4 * qc
                            nc.vector.tensor_add(s_ps[:], s_ps[:], mask_sb[:, j, :])
                        p_sb = pexp.tile([128, SC], BF16, tag="p")
                        nc.scalar.activation(out=p_sb[:], in_=s_ps[:], func=AF.Exp)
                        nc.tensor.matmul(
                            o_ps[:],
                            lhsT=v_sb[:, kt, :],
                            rhs=p_sb[:],
                            start=(kt == 0),
                            stop=(kt == nkt - 1),
                        )
                        nc.tensor.matmul(
                            l_ps[:],
                            lhsT=ones[:],
                            rhs=p_sb[:],
                            start=(kt == 0),
                            stop=(kt == nkt - 1),
                        )
                    # l broadcast across partitions via K=1 matmul, then
                    # reciprocal at full lane width and normalize.
                    l_sb = small.tile([1, SC], BF16, tag="lsb")
                    nc.scalar.activation(out=l_sb[:], in_=l_ps[:], func=AF.Copy)
                    lb = acc.tile([128, SC], F32, tag="acc")
                    nc.tensor.matmul(
                        lb[:], lhsT=onesr[:], rhs=l_sb[:], start=True, stop=True
                    )
                    rb = small.tile([128, SC], F32, tag="rb")
                    nc.vector.reciprocal(out=rb[:], in_=lb[:])
                    nc.vector.tensor_mul(onrm[:, h, qsl], o_ps[:], rb[:])

            # ---- phase 3: output projection (partial over this group) ----
            for st in range(S // 128):
                stl = slice(st * 128, (st + 1) * 128)
                for ec in range(E // SC):
                    esl = slice(ec * SC, (ec + 1) * SC)
                    ps = acc.tile([128, SC], F32, tag="acc")
                    for h in range(G):
                        nc.tensor.matmul(
                            ps[:],
                            lhsT=onrm[:, h, stl],
                            rhs=wo_sb[:, h, esl],
                            start=(h == 0),
                            stop=(h == G - 1),
                        )
                    ob = obp.tile([128, SC], F32, tag="ob")
                    nc.scalar.activation(out=ob[:], in_=ps[:], func=AF.Copy)
                    nc.sync.dma_start(out=out[stl, esl], in_=ob[:])
    nc.compile()
    return nc


_NC_CACHE = None


def _get_nc():
    global _NC_CACHE
    if _NC_CACHE is None:
        _NC_CACHE = build_nc()
    return _NC_CACHE


def _prep_in_maps(query, key, value, attn_mask, Wq, Wk, Wv, Wo):
    query = np.asarray(query, dtype=np.float32)
    key = np.asarray(key, dtype=np.float32)
    value = np.asarray(value, dtype=np.float32)
    Wq = np.asarray(Wq, dtype=np.float32)
    Wk = np.asarray(Wk, dtype=np.float32)
    Wv = np.asarray(Wv, dtype=np.float32)
    Wo = np.asarray(Wo, dtype=np.float32)
    am = np.asarray(attn_mask)

    xqT = [np.ascontiguousarray(query[b].T).astype(NPBF) for b in range(B)]
    xkT = [np.ascontiguousarray(key[b].T).astype(NPBF) for b in range(B)]
    xvT = [np.ascontiguousarray(value[b].T).astype(NPBF) for b in range(B)]

    # 4 diagonal mask tiles [128, SC]: tile j covers k in [j*128,(j+1)*128)
    # relative to the q-chunk start; additive -1e9 on masked entries.
    m0 = np.asarray(am[0, 0, :SC, :SC], dtype=np.float32)  # [q, k] for chunk 0
    msk_tiles = np.zeros((4 * 128, SC), dtype=np.float32)
    for j in range(4):
        msk_tiles[j * 128 : (j + 1) * 128, :] = (
            m0[:, j * 128 : (j + 1) * 128].T - 1.0
        ) * 1e9
    in_maps = []
    for b in range(B):
        for g in range(HKV):
            in_maps.append(
                {
                    "xq": xqT[b],
                    "xk": xkT[b],
                    "xv": xvT[b],
                    "wq": np.ascontiguousarray(
                        Wq[g * GD : (g + 1) * GD, :].T
                    ).astype(NPBF),
                    "wk": np.ascontiguousarray(
                        Wk[g * D : (g + 1) * D, :].T
                    ).astype(NPBF),
                    "wv": np.ascontiguousarray(
                        Wv[g * D : (g + 1) * D, :].T
                    ).astype(NPBF),
                    "wo": np.ascontiguousarray(
                        Wo[:, g * GD : (g + 1) * GD].T
                    ).astype(NPBF),
                    "msk": msk_tiles,
                }
            )
    return in_maps


def _run(inputs, trace=False, **kw):
    nc = _get_nc()
    in_maps = _prep_in_maps(**inputs)
    res = run_bass_kernel_spmd(
        nc, in_maps, list(range(NCORES)), trace=trace, **kw
    )
    outs = [np.asarray(r["out"]) for r in res.results]
    full = np.empty((B, S, E), dtype=np.float32)
    for b in range(B):
        acc = outs[b * HKV].astype(np.float32)
        for g in range(1, HKV):
            acc = acc + outs[b * HKV + g]
        full[b] = acc
    return full, res


def kernel(**inputs):
    full, _ = _run(inputs, trace=False)
    return full


# revision 10
# speedup vs baseline: 1.0996x; 1.0996x over previous
"""GQA kernel for Trainium2, 8 NeuronCores.

Sharding: core c = b*4 + g  handles batch b, kv-head g (4 query heads).
Each core computes:
  Q_g^T = Wq_g @ x_q^T        [4 heads][128, S]   (scale 1/sqrt(D) folded in)
  K_g^T = Wk_g @ x_k^T        [128, S]
  V_g   = (x_v @ Wv_g.T)      [S, 128]  (via V^T then PE transpose)
  S^T   = K tile @ Q^T        [k,q] orientation -> +mask (diag) -> exp
  o^T  += V[kt] matmul P~^T   (PSUM accum), l += ones^T P~^T
  o_norm^T = o^T * recip(bcast l)
  partial = o_norm @ Wo_g.T   [S, E]
Host sums the 4 partials per batch.

Matmuls run in bf16 (fp32 PSUM accumulation): 4-byte dtypes serialize
LDWEIGHTS with the matmul (~191ns per 128x128 load, no FWL/prefetch),
which was ~37% of the kernel span in fp32r. l is broadcast across
partitions with a K=1 matmul so the reciprocal runs at full DVE lane
width ([128,512] not [1,512]).
"""

import sys

import numpy as np

for _p in ("/opt/trn_rl_repo",):
    if _p not in sys.path:
        sys.path.insert(0, _p)

import ml_dtypes

import concourse.bass as bass
import concourse.mybir as mybir
from concourse import bacc
from concourse.bass_utils import run_bass_kernel_spmd
from concourse.masks import make_identity
from concourse.tile import TileContext

B, S, E = 2, 2048, 2048
H, HKV = 16, 4
D = E // H  # 128
G = H // HKV  # 4 query heads per kv head
GD = G * D  # 512
NCORES = B * HKV  # 8
SC = 512  # s/q chunk width (free dim of matmuls)
NSC = S // SC  # 4
NET = E // 128  # 16 e-tiles (contraction)
NKT = S // 128  # 16 k-tiles
SCALE = 1.0 / float(np.sqrt(D))

F32 = mybir.dt.float32
BF16 = mybir.dt.bfloat16
F32R = mybir.dt.float32r
AF = mybir.ActivationFunctionType
NPBF = np.dtype(ml_dtypes.bfloat16)


def build_nc():
    nc = bacc.Bacc()
    xq = nc.declare_dram_parameter("xq", [E, S], BF16, isOutput=False)  # query[b].T
    xk = nc.declare_dram_parameter("xk", [E, S], BF16, isOutput=False)  # key[b].T
    xv = nc.declare_dram_parameter("xv", [E, S], BF16, isOutput=False)  # value[b].T
    wq = nc.declare_dram_parameter("wq", [E, GD], BF16, isOutput=False)
    wk = nc.declare_dram_parameter("wk", [E, D], BF16, isOutput=False)
    wv = nc.declare_dram_parameter("wv", [E, D], BF16, isOutput=False)
    wo = nc.declare_dram_parameter("wo", [GD, E], BF16, isOutput=False)
    msk = nc.declare_dram_parameter("msk", [4 * 128, SC], F32, isOutput=False)
    out = nc.declare_dram_parameter("out", [S, E], F32, isOutput=True)

    with TileContext(nc) as tc:
        with (
            tc.tile_pool(name="singles", bufs=1) as singles,
            tc.tile_pool(name="xt", bufs=4) as xtp,
            tc.tile_pool(name="pexp", bufs=4) as pexp,
            tc.tile_pool(name="small", bufs=2) as small,
            tc.tile_pool(name="ob", bufs=3) as obp,
            tc.tile_pool(name="acc", bufs=4, space="PSUM") as acc,
            tc.tile_pool(name="ops", bufs=2, space="PSUM") as ops,
            tc.tile_pool(name="lps", bufs=1, space="PSUM") as lps,
            tc.tile_pool(name="trp", bufs=1, space="PSUM") as trp,
            tc.tile_pool(name="drp", bufs=2, space="DRAM") as drp,
        ):
            # ---- constants / weights resident in SBUF ----
            wq_sb = singles.tile([128, NET, GD], BF16)  # 16KB/p
            wk_sb = singles.tile([128, NET, D], BF16)  # 4KB/p
            wv_sb = singles.tile([128, NET, D], BF16)  # 4KB/p
            wo_sb = singles.tile([128, G, E], BF16)  # 16KB/p
            mask_sb = singles.tile([128, 4, SC], F32)  # 8KB/p
            ident_f = singles.tile([128, 128], F32)
            ident = singles.tile([128, 128], BF16)
            ones_f = singles.tile([128, 1], F32)
            ones = singles.tile([128, 1], BF16)
            qT = singles.tile([128, G, S], BF16)  # 16KB/p
            kT = singles.tile([128, S], BF16)  # 4KB/p
            v_sb = singles.tile([128, NKT, D], BF16)  # 4KB/p
            onrm = singles.tile([128, G, S], BF16)  # 16KB/p
            o_unn = singles.tile([128, G, S], F32)  # 32KB/p

            make_identity(nc, ident_f)
            nc.scalar.activation(out=ident[:], in_=ident_f[:], func=AF.Copy)
            nc.vector.memset(ones_f, 1.0)
            nc.scalar.activation(out=ones[:], in_=ones_f[:], func=AF.Copy)
            for t in range(NET):
                nc.sync.dma_start(out=wq_sb[:, t, :], in_=wq[t * 128 : (t + 1) * 128, :])
                nc.sync.dma_start(out=wk_sb[:, t, :], in_=wk[t * 128 : (t + 1) * 128, :])
                nc.sync.dma_start(out=wv_sb[:, t, :], in_=wv[t * 128 : (t + 1) * 128, :])
            for h in range(G):
                nc.sync.dma_start(
                    out=wo_sb[:, h, :], in_=wo[h * 128 : (h + 1) * 128, :]
                )
            for j in range(4):
                nc.sync.dma_start(
                    out=mask_sb[:, j, :], in_=msk[j * 128 : (j + 1) * 128, :]
                )

            # ---- phase 1: projections ----
            for sc in range(NSC):
                ssl = slice(sc * SC, (sc + 1) * SC)
                # Q^T: 4 heads
                xts = []
                for t in range(NET):
                    xt = xtp.tile([128, SC], BF16, tag="xt")
                    nc.sync.dma_start(out=xt, in_=xq[t * 128 : (t + 1) * 128, ssl])
                    xts.append(xt)
                for h in range(G):
                    ps = acc.tile([128, SC], F32, tag="acc")
                    for t in range(NET):
                        nc.tensor.matmul(
                            ps[:],
                            lhsT=wq_sb[:, t, h * D : (h + 1) * D],
                            rhs=xts[t][:],
                            start=(t == 0),
                            stop=(t == NET - 1),
                        )
                    # fold softmax scale into Q
                    nc.scalar.activation(
                        out=qT[:, h, ssl], in_=ps[:], func=AF.Copy, scale=SCALE
                    )
                # K^T
                xts = []
                for t in range(NET):
                    xt = xtp.tile([128, SC], BF16, tag="xt")
                    nc.sync.dma_start(out=xt, in_=xk[t * 128 : (t + 1) * 128, ssl])
                    xts.append(xt)
                ps = acc.tile([128, SC], F32, tag="acc")
                for t in range(NET):
                    nc.tensor.matmul(
                        ps[:],
                        lhsT=wk_sb[:, t, :],
                        rhs=xts[t][:],
                        start=(t == 0),
                        stop=(t == NET - 1),
                    )
                nc.vector.tensor_copy(out=kT[:, ssl], in_=ps[:])
                # V^T then transpose to V [s, d]
                xts = []
                for t in range(NET):
                    xt = xtp.tile([128, SC], BF16, tag="xt")
                    nc.sync.dma_start(out=xt, in_=xv[t * 128 : (t + 1) * 128, ssl])
                    xts.append(xt)
                ps = acc.tile([128, SC], F32, tag="acc")
                for t in range(NET):
                    nc.tensor.matmul(
                        ps[:],
                        lhsT=wv_sb[:, t, :],
                        rhs=xts[t][:],
                        start=(t == 0),
                        stop=(t == NET - 1),
                    )
                vt_tmp = small.tile([128, SC], BF16, tag="vt")
                nc.scalar.activation(out=vt_tmp[:], in_=ps[:], func=AF.Copy)
                for i in range(SC // 128):
                    tp = trp.tile([128, 128], BF16, tag="tr")
                    nc.tensor.transpose(
                        tp[:], vt_tmp[:, i * 128 : (i + 1) * 128], ident[:]
                    )
                    nc.vector.tensor_copy(out=v_sb[:, sc * 4 + i, :], in_=tp[:])

            # ---- phase 2: attention (flash, S^T orientation) ----
            for h in range(G):
                for qc in range(NSC):
                    qsl = slice(qc * SC, (qc + 1) * SC)
                    nkt = (qc + 1) * (SC // 128)  # causal: k tiles 0..nkt-1
                    o_ps = ops.tile([128, SC], F32, tag="o")
                    l_ps = lps.tile([1, SC], F32, tag="l")
                    for kt in range(nkt):
                        s_ps = acc.tile([128, SC], F32, tag="acc")
                        nc.tensor.matmul(
                            s_ps[:],
                            lhsT=kT[:, kt * 128 : (kt + 1) * 128],
                            rhs=qT[:, h, qsl],
                            start=True,
                            stop=True,
                        )
                        if kt >= nkt - 4:
                            j = kt - 4 * qc
                            nc.vector.tensor_add(s_ps[:], s_ps[:], mask_sb[:, j, :])
                        p_sb = pexp.tile([128, SC], BF16, tag="p")
                        nc.scalar.activation(out=p_sb[:], in_=s_ps[:], func=AF.Exp)
                        nc.tensor.matmul(
                            o_ps[:],
                            lhsT=v_sb[:, kt, :],
                            rhs=p_sb[:],
                            start=(kt == 0),
                            stop=(kt == nkt - 1),
                        )
                        nc.tensor.matmul(
                            l_ps[:],
                            lhsT=ones[:],
                            rhs=p_sb[:],
                            start=(kt == 0),
                            stop=(kt == nkt - 1),
                        )
                    # l broadcast across partitions via K=1 matmul, then
                    # reciprocal at full lane width and normalize.
                    nc.scalar.activation(
                        out=o_unn[:, h, qsl], in_=o_ps[:], func=AF.Copy
                    )
                    l_sb = small.tile([1, SC], F32, tag="lsb")
                    nc.scalar.activation(out=l_sb[:], in_=l_ps[:], func=AF.Copy)
                    l_dr = drp.tile([1, SC], F32, tag="ldr")
                    nc.sync.dma_start(out=l_dr[:], in_=l_sb[:])
                    lb = small.tile([128, SC], F32, tag="lb")
                    l_bc = bass.AP(
                        tensor=l_dr[:].tensor,
                        offset=l_dr[:].offset,
                        ap=[[0, 128]] + list(l_dr[:].ap[1:]),
                    )
                    nc.sync.dma_start(out=lb[:], in_=l_bc)
                    rb = small.tile([128, SC], F32, tag="rb")
                    nc.vector.reciprocal(out=rb[:], in_=lb[:])
                    nc.vector.tensor_mul(
                        onrm[:, h, qsl], o_unn[:, h, qsl], rb[:]
                    )

            # ---- phase 3: output projection (partial over this group) ----
            for st in range(S // 128):
                stl = slice(st * 128, (st + 1) * 128)
                for ec in range(E // SC):
                    esl = slice(ec * SC, (ec + 1) * SC)
                    ps = acc.tile([128, SC], F32, tag="acc")
                    for h in range(G):
                        nc.tensor.matmul(
                            ps[:],
                            lhsT=onrm[:, h, stl],
                            rhs=wo_sb[:, h, esl],
                            start=(h == 0),
                            stop=(h == G - 1),
                        )
                    ob = obp.tile([128, SC], F32, tag="ob")
                    nc.scalar.activation(out=ob[:], in_=ps[:], func=AF.Copy)
                    nc.sync.dma_start(out=out[stl, esl], in_=ob[:])
    nc.compile()
    return nc


_NC_CACHE = None


def _get_nc():
    global _NC_CACHE
    if _NC_CACHE is None:
        _NC_CACHE = build_nc()
    return _NC_CACHE


def _prep_in_maps(query, key, value, attn_mask, Wq, Wk, Wv, Wo):
    query = np.asarray(query, dtype=np.float32)
    key = np.asarray(key, dtype=np.float32)
    value = np.asarray(value, dtype=np.float32)
    Wq = np.asarray(Wq, dtype=np.float32)
    Wk = np.asarray(Wk, dtype=np.float32)
    Wv = np.asarray(Wv, dtype=np.float32)
    Wo = np.asarray(Wo, dtype=np.float32)
    am = np.asarray(attn_mask)

    xqT = [np.ascontiguousarray(query[b].T).astype(NPBF) for b in range(B)]
    xkT = [np.ascontiguousarray(key[b].T).astype(NPBF) for b in range(B)]
    xvT = [np.ascontiguousarray(value[b].T).astype(NPBF) for b in range(B)]

    # 4 diagonal mask tiles [128, SC]: tile j covers k in [j*128,(j+1)*128)
    # relative to the q-chunk start; additive -1e9 on masked entries.
    m0 = np.asarray(am[0, 0, :SC, :SC], dtype=np.float32)  # [q, k] for chunk 0
    msk_tiles = np.zeros((4 * 128, SC), dtype=np.float32)
    for j in range(4):
        msk_tiles[j * 128 : (j + 1) * 128, :] = (
            m0[:, j * 128 : (j + 1) * 128].T - 1.0
        ) * 1e9
    in_maps = []
    for b in range(B):
        for g in range(HKV):
            in_maps.append(
                {
                    "xq": xqT[b],
                    "xk": xkT[b],
                    "xv": xvT[b],
                    "wq": np.ascontiguousarray(
                        Wq[g * GD : (g + 1) * GD, :].T
                    ).astype(NPBF),
                    "wk": np.ascontiguousarray(
                        Wk[g * D : (g + 1) * D, :].T
                    ).astype(NPBF),
                    "wv": np.ascontiguousarray(
                        Wv[g * D : (g + 1) * D, :].T
                    ).astype(NPBF),
                    "wo": np.ascontiguousarray(
                        Wo[:, g * GD : (g + 1) * GD].T
                    ).astype(NPBF),
                    "msk": msk_tiles,
                }
            )
    return in_maps


def _run(inputs, trace=False, **kw):
    nc = _get_nc()
    in_maps = _prep_in_maps(**inputs)
    res = run_bass_kernel_spmd(
        nc, in_maps, list(range(NCORES)), trace=trace, **kw
    )
    outs = [np.asarray(r["out"]) for r in res.results]
    full = np.empty((B, S, E), dtype=np.float32)
    for b in range(B):
        acc = outs[b * HKV].astype(np.float32)
        for g in range(1, HKV):
            acc = acc + outs[b * HKV + g]
        full[b] = acc
    return full, res


def kernel(**inputs):
    full, _ = _run(inputs, trace=False)
    return full


# revision 13
# speedup vs baseline: 1.2797x; 1.1637x over previous
"""GQA kernel for Trainium2, 8 NeuronCores.

Sharding: core c = b*4 + g  handles batch b, kv-head g (4 query heads).
Each core computes:
  Q_g^T = Wq_g @ x_q^T        [4 heads][128, S]   (scale 1/sqrt(D) folded in)
  K_g^T = Wk_g @ x_k^T        [128, S]
  V_g   = (x_v @ Wv_g.T)      [S, 128]  (via V^T then PE transpose)
  S^T   = K tile @ Q^T        [k,q] orientation -> +mask (diag) -> exp
  o^T  += V[kt] matmul P~^T   (PSUM accum), l += ones^T P~^T
  o_norm^T = o^T * recip(bcast l)
  partial = o_norm @ Wo_g.T   [S, E]
Host sums the 4 partials per batch.

Matmuls run in bf16 (fp32 PSUM accumulation): 4-byte dtypes serialize
LDWEIGHTS with the matmul (~191ns per 128x128 load, no FWL/prefetch),
which was ~37% of the kernel span in fp32r. l is broadcast across
partitions with a K=1 matmul so the reciprocal runs at full DVE lane
width ([128,512] not [1,512]).
"""

import sys

import numpy as np

for _p in ("/opt/trn_rl_repo",):
    if _p not in sys.path:
        sys.path.insert(0, _p)

import ml_dtypes

import concourse.bass as bass
import concourse.mybir as mybir
from concourse import bacc
from concourse.bass_utils import run_bass_kernel_spmd
from concourse.masks import make_identity
from concourse.tile import TileContext

B, S, E = 2, 2048, 2048
H, HKV = 16, 4
D = E // H  # 128
G = H // HKV  # 4 query heads per kv head
GD = G * D  # 512
NCORES = B * HKV  # 8
SC = 512  # s/q chunk width (free dim of matmuls)
NSC = S // SC  # 4
NET = E // 128  # 16 e-tiles (contraction)
NKT = S // 128  # 16 k-tiles
SCALE = 1.0 / float(np.sqrt(D))

F32 = mybir.dt.float32
BF16 = mybir.dt.bfloat16
F32R = mybir.dt.float32r
AF = mybir.ActivationFunctionType
NPBF = np.dtype(ml_dtypes.bfloat16)


def build_nc():
    nc = bacc.Bacc()
    xq = nc.declare_dram_parameter("xq", [E, S], BF16, isOutput=False)  # query[b].T
    xk = nc.declare_dram_parameter("xk", [E, S], BF16, isOutput=False)  # key[b].T
    xv = nc.declare_dram_parameter("xv", [E, S], BF16, isOutput=False)  # value[b].T
    wq = nc.declare_dram_parameter("wq", [E, GD], BF16, isOutput=False)
    wk = nc.declare_dram_parameter("wk", [E, D], BF16, isOutput=False)
    wv = nc.declare_dram_parameter("wv", [E, D], BF16, isOutput=False)
    wo = nc.declare_dram_parameter("wo", [GD, E], BF16, isOutput=False)
    msk = nc.declare_dram_parameter("msk", [4 * 128, SC], F32, isOutput=False)
    out = nc.declare_dram_parameter("out", [S, E], F32, isOutput=True)

    with TileContext(nc) as tc:
        with (
            tc.tile_pool(name="singles", bufs=1) as singles,
            tc.tile_pool(name="xt", bufs=24) as xtp,
            tc.tile_pool(name="pexp", bufs=4) as pexp,
            tc.tile_pool(name="small", bufs=2) as small,
            tc.tile_pool(name="ob", bufs=3) as obp,
            tc.tile_pool(name="acc", bufs=4, space="PSUM") as acc,
            tc.tile_pool(name="ops", bufs=2, space="PSUM") as ops,
            tc.tile_pool(name="lps", bufs=1, space="PSUM") as lps,
            tc.tile_pool(name="trp", bufs=1, space="PSUM") as trp,
            tc.tile_pool(name="drp", bufs=2, space="DRAM") as drp,
        ):
            # ---- constants / weights resident in SBUF ----
            wq_sb = singles.tile([128, NET, GD], BF16)  # 16KB/p
            wk_sb = singles.tile([128, NET, D], BF16)  # 4KB/p
            wv_sb = singles.tile([128, NET, D], BF16)  # 4KB/p
            wo_sb = singles.tile([128, G, E], BF16)  # 16KB/p
            mask_sb = singles.tile([128, 4, SC], F32)  # 8KB/p
            ident_f = singles.tile([128, 128], F32)
            ident = singles.tile([128, 128], BF16)
            ones_f = singles.tile([128, 1], F32)
            ones = singles.tile([128, 1], BF16)
            qT = singles.tile([128, G, S], BF16)  # 16KB/p
            kT = singles.tile([128, S], BF16)  # 4KB/p
            v_sb = singles.tile([128, NKT, D], BF16)  # 4KB/p
            onrm = singles.tile([128, G, S], BF16)  # 16KB/p
            o_unn = singles.tile([128, G, S], F32)  # 32KB/p

            make_identity(nc, ident_f)
            nc.scalar.activation(out=ident[:], in_=ident_f[:], func=AF.Copy)
            nc.vector.memset(ones_f, 1.0)
            nc.scalar.activation(out=ones[:], in_=ones_f[:], func=AF.Copy)
            for t in range(NET):
                nc.sync.dma_start(
                    out=wq_sb[:, t, :], in_=wq[t * 128 : (t + 1) * 128, :]
                )
                nc.sync.dma_start(out=wk_sb[:, t, :], in_=wk[t * 128 : (t + 1) * 128, :])
                nc.sync.dma_start(out=wv_sb[:, t, :], in_=wv[t * 128 : (t + 1) * 128, :])
            for h in range(G):
                nc.sync.dma_start(
                    out=wo_sb[:, h, :], in_=wo[h * 128 : (h + 1) * 128, :]
                )
            for j in range(4):
                nc.sync.dma_start(
                    out=mask_sb[:, j, :], in_=msk[j * 128 : (j + 1) * 128, :]
                )

            # ---- phase 1: projections ----
            for sc in range(NSC):
                ssl = slice(sc * SC, (sc + 1) * SC)
                # Q^T: 4 heads
                xts = []
                for t in range(NET):
                    xt = xtp.tile([128, SC], BF16, tag="xt")
                    nc.sync.dma_start(out=xt, in_=xq[t * 128 : (t + 1) * 128, ssl])
                    xts.append(xt)
                for h in range(G):
                    ps = acc.tile([128, SC], F32, tag="acc")
                    for t in range(NET):
                        nc.tensor.matmul(
                            ps[:],
                            lhsT=wq_sb[:, t, h * D : (h + 1) * D],
                            rhs=xts[t][:],
                            start=(t == 0),
                            stop=(t == NET - 1),
                        )
                    # fold softmax scale into Q
                    nc.scalar.activation(
                        out=qT[:, h, ssl], in_=ps[:], func=AF.Copy, scale=SCALE
                    )
                # K^T
                xts = []
                for t in range(NET):
                    xt = xtp.tile([128, SC], BF16, tag="xt")
                    nc.sync.dma_start(out=xt, in_=xk[t * 128 : (t + 1) * 128, ssl])
                    xts.append(xt)
                ps = acc.tile([128, SC], F32, tag="acc")
                for t in range(NET):
                    nc.tensor.matmul(
                        ps[:],
                        lhsT=wk_sb[:, t, :],
                        rhs=xts[t][:],
                        start=(t == 0),
                        stop=(t == NET - 1),
                    )
                nc.vector.tensor_copy(out=kT[:, ssl], in_=ps[:])
                # V^T then transpose to V [s, d]
                xts = []
                for t in range(NET):
                    xt = xtp.tile([128, SC], BF16, tag="xt")
                    nc.sync.dma_start(out=xt, in_=xv[t * 128 : (t + 1) * 128, ssl])
                    xts.append(xt)
                ps = acc.tile([128, SC], F32, tag="acc")
                for t in range(NET):
                    nc.tensor.matmul(
                        ps[:],
                        lhsT=wv_sb[:, t, :],
                        rhs=xts[t][:],
                        start=(t == 0),
                        stop=(t == NET - 1),
                    )
                vt_tmp = small.tile([128, SC], BF16, tag="vt")
                nc.scalar.activation(out=vt_tmp[:], in_=ps[:], func=AF.Copy)
                for i in range(SC // 128):
                    tp = trp.tile([128, 128], BF16, tag="tr")
                    nc.tensor.transpose(
                        tp[:], vt_tmp[:, i * 128 : (i + 1) * 128], ident[:]
                    )
                    nc.vector.tensor_copy(out=v_sb[:, sc * 4 + i, :], in_=tp[:])

            # ---- phase 2+3: attention, outproj interleaved per q-chunk ----
            for qc in range(NSC):
                for h in range(G):
                    qsl = slice(qc * SC, (qc + 1) * SC)
                    nkt = (qc + 1) * (SC // 128)  # causal: k tiles 0..nkt-1
                    o_ps = ops.tile([128, SC], F32, tag="o")
                    l_ps = lps.tile([1, SC], F32, tag="l")
                    for kt in range(nkt):
                        s_ps = acc.tile([128, SC], F32, tag="acc")
                        nc.tensor.matmul(
                            s_ps[:],
                            lhsT=kT[:, kt * 128 : (kt + 1) * 128],
                            rhs=qT[:, h, qsl],
                            start=True,
                            stop=True,
                        )
                        if kt >= nkt - 4:
                            j = kt - 4 * qc
                            nc.vector.tensor_add(s_ps[:], s_ps[:], mask_sb[:, j, :])
                        p_sb = pexp.tile([128, SC], BF16, tag="p")
                        nc.scalar.activation(out=p_sb[:], in_=s_ps[:], func=AF.Exp)
                        nc.tensor.matmul(
                            o_ps[:],
                            lhsT=v_sb[:, kt, :],
                            rhs=p_sb[:],
                            start=(kt == 0),
                            stop=(kt == nkt - 1),
                        )
                        nc.tensor.matmul(
                            l_ps[:],
                            lhsT=ones[:],
                            rhs=p_sb[:],
                            start=(kt == 0),
                            stop=(kt == nkt - 1),
                        )
                    # l broadcast across partitions via K=1 matmul, then
                    # reciprocal at full lane width and normalize.
                    nc.scalar.activation(
                        out=o_unn[:, h, qsl], in_=o_ps[:], func=AF.Copy
                    )
                    l_sb = small.tile([1, SC], F32, tag="lsb")
                    nc.scalar.activation(out=l_sb[:], in_=l_ps[:], func=AF.Copy)
                    l_dr = drp.tile([1, SC], F32, tag="ldr")
                    nc.sync.dma_start(out=l_dr[:], in_=l_sb[:])
                    lb = small.tile([128, SC], F32, tag="lb")
                    l_bc = bass.AP(
                        tensor=l_dr[:].tensor,
                        offset=l_dr[:].offset,
                        ap=[[0, 128]] + list(l_dr[:].ap[1:]),
                    )
                    nc.sync.dma_start(out=lb[:], in_=l_bc)
                    rb = small.tile([128, SC], F32, tag="rb")
                    nc.vector.reciprocal(out=rb[:], in_=lb[:])
                    nc.vector.tensor_mul(
                        onrm[:, h, qsl], o_unn[:, h, qsl], rb[:]
                    )

                # output projection for this q-chunk's 4 s-tiles
                for sti in range(SC // 128):
                    st = qc * (SC // 128) + sti
                    stl = slice(st * 128, (st + 1) * 128)
                    for ec in range(E // SC):
                        esl = slice(ec * SC, (ec + 1) * SC)
                        ps = acc.tile([128, SC], F32, tag="acc")
                        for h in range(G):
                            nc.tensor.matmul(
                                ps[:],
                                lhsT=onrm[:, h, stl],
                                rhs=wo_sb[:, h, esl],
                                start=(h == 0),
                                stop=(h == G - 1),
                            )
                        ob = obp.tile([128, SC], F32, tag="ob")
                        nc.scalar.activation(out=ob[:], in_=ps[:], func=AF.Copy)
                        nc.sync.dma_start(out=out[stl, esl], in_=ob[:])
    nc.compile()
    return nc


_NC_CACHE = None


def _get_nc():
    global _NC_CACHE
    if _NC_CACHE is None:
        _NC_CACHE = build_nc()
    return _NC_CACHE


def _prep_in_maps(query, key, value, attn_mask, Wq, Wk, Wv, Wo):
    query = np.asarray(query, dtype=np.float32)
    key = np.asarray(key, dtype=np.float32)
    value = np.asarray(value, dtype=np.float32)
    Wq = np.asarray(Wq, dtype=np.float32)
    Wk = np.asarray(Wk, dtype=np.float32)
    Wv = np.asarray(Wv, dtype=np.float32)
    Wo = np.asarray(Wo, dtype=np.float32)
    am = np.asarray(attn_mask)

    xqT = [np.ascontiguousarray(query[b].T).astype(NPBF) for b in range(B)]
    xkT = [np.ascontiguousarray(key[b].T).astype(NPBF) for b in range(B)]
    xvT = [np.ascontiguousarray(value[b].T).astype(NPBF) for b in range(B)]

    # 4 diagonal mask tiles [128, SC]: tile j covers k in [j*128,(j+1)*128)
    # relative to the q-chunk start; additive -1e9 on masked entries.
    m0 = np.asarray(am[0, 0, :SC, :SC], dtype=np.float32)  # [q, k] for chunk 0
    msk_tiles = np.zeros((4 * 128, SC), dtype=np.float32)
    for j in range(4):
        msk_tiles[j * 128 : (j + 1) * 128, :] = (
            m0[:, j * 128 : (j + 1) * 128].T - 1.0
        ) * 1e9
    in_maps = []
    for b in range(B):
        for g in range(HKV):
            in_maps.append(
                {
                    "xq": xqT[b],
                    "xk": xkT[b],
                    "xv": xvT[b],
                    "wq": np.ascontiguousarray(
                        Wq[g * GD : (g + 1) * GD, :].T
                    ).astype(NPBF),
                    "wk": np.ascontiguousarray(
                        Wk[g * D : (g + 1) * D, :].T
                    ).astype(NPBF),
                    "wv": np.ascontiguousarray(
                        Wv[g * D : (g + 1) * D, :].T
                    ).astype(NPBF),
                    "wo": np.ascontiguousarray(
                        Wo[:, g * GD : (g + 1) * GD].T
                    ).astype(NPBF),
                    "msk": msk_tiles,
                }
            )
    return in_maps


def _run(inputs, trace=False, **kw):
    nc = _get_nc()
    in_maps = _prep_in_maps(**inputs)
    res = run_bass_kernel_spmd(
        nc, in_maps, list(range(NCORES)), trace=trace, **kw
    )
    outs = [np.asarray(r["out"]) for r in res.results]
    full = np.empty((B, S, E), dtype=np.float32)
    for b in range(B):
        acc = outs[b * HKV].astype(np.float32)
        for g in range(1, HKV):
            acc = acc + outs[b * HKV + g]
        full[b] = acc
    return full, res


def kernel(**inputs):
    full, _ = _run(inputs, trace=False)
    return full
